# revision 7
# baseline (speedup 1.0000x reference)
# DenseGATConv on 8 Trainium2 NeuronCores (Bass/Tile, SPMD over destination rows).
#
# Math: h = x@W ; el/er = head-wise <h, att> ; e_ij = leaky(el_i + er_j) ;
#       alpha = softmax_j(mask(e)) ; out_i = sum_j alpha_ij h_j + bias.
# Key identity: exp(leaky(s)) = max(exp(s), exp(0.2 s)), so the masked
# unnormalized attention splits as pm = m*A + m*relu(B - A) with
# A = al_i ar_j and B = bl_i br_j rank-1 (al=exp(el), ar=exp(er), etc).
# The A part aggregates as adjT^T @ (ar*h65) (matmuls end to end), the
# residual needs one elementwise relu+mask pass over [N, NB, H].
#
# Schedule (vs the first working kernel; ~155us -> ~135us):
#  - h65 and P1 run as ONE interleaved loop (P1 deferred 6 tiles, arh
#    scaled just-in-time from SBUF - PSUM-read elementwise pays ~2x).
#  - all PE matmuls are pinned to emission order via ordering-only deps
#    (add_dep_helper): Tile's list scheduler otherwise emits
#    engine-starving schedules (e.g. the whole Y side before any X).
#  - main loop is unrolled x2 with d pairs pre-emitted 2 iterations
#    ahead and BOTH aggregations deferred so far that their semaphore
#    waits are covered by the d pair's fresher waits; aggregations are
#    flushed in batches of 8 back-to-back matmuls (a run of wait-free
#    matmuls keeps the PE fill/drain pipeline unbroken: 427ns -> ~215ns
#    per 512-col matmul).  In-pair emission is newest-first so one wait
#    covers the batch.
#  - elementwise split per jt: head Y fused relu+mask STT on DVE; head X
#    relu on ACT then mask on GpSimd (bf16 mult) or DVE (bitwise AND on
#    packed bf16 pairs, every 4th tile) - balances all three engines.
#  - transpose/normalize epilogue at the tail via PE transposes.
import numpy as np

N, IN_C, HEADS, OUT_C = 4096, 256, 4, 64
HC = HEADS * OUT_C          # 256
NCORES = 8
NB = N // NCORES            # 512 destination rows per core
JT = N // 128               # 32 source-node tiles
IT = NB // 128              # 4 row subtiles per core
C65 = OUT_C + 1             # head slice + ar column
WC = HC + HEADS             # W cols + War cols

TRACE = False               # test.py flips this to collect HW exec time
LAST_RESULTS = {}           # exec_time_ns etc. stashed here when TRACE

_compiled = {}


def _emit(ctx, tc, nc, io):
    import concourse.bass as bass
    import concourse.masks as masks
    from concourse import mybir
    from concourse.tile import add_dep_helper

    dt = mybir.dt
    Alu = mybir.AluOpType
    Act = mybir.ActivationFunctionType

    xT, xoT, adjbT, Waug, Wal, bias, out = (
        io["xT"], io["xoT"], io["adjbT"], io["Waug"], io["Wal"],
        io["bias"], io["out"],
    )
    adjM = io["adjmask"]

    big = ctx.enter_context(tc.tile_pool(name="big", bufs=1))
    rYp = ctx.enter_context(tc.tile_pool(name="rYp", bufs=10))
    rXp = ctx.enter_context(tc.tile_pool(name="rXp", bufs=12))
    rdp = ctx.enter_context(tc.tile_pool(name="rdp", bufs=6))
    arhp = ctx.enter_context(tc.tile_pool(name="arhp", bufs=9))
    tr = ctx.enter_context(tc.tile_pool(name="tr", bufs=3))

    pe_prev = [None]

    def pe_mm(mm):
        # pin the PE queue to emission order (ordering-only edge): the
        # list scheduler otherwise builds engine-starving schedules
        if pe_prev[0] is not None:
            add_dep_helper(mm.ins, pe_prev[0].ins, sync=False,
                           reason="pe-order")
        pe_prev[0] = mm

    # ---- constants / params -------------------------------------------------
    idb = big.tile([128, 128], dt.bfloat16, tag="idb")
    masks.make_identity(nc, idb[:])
    bias_b = big.tile([128, HC], dt.float32, tag="bias_b")
    bias_bcast_ap = bass.AP(
        tensor=bias.tensor, offset=bias.offset, ap=[[0, 128]] + list(bias.ap)
    )
    nc.gpsimd.dma_start(out=bias_b[:], in_=bias_bcast_ap)

    # everything rides the sync queue (its engine has no other work; the
    # scalar queue must stay free for ACT); each dma_start costs ~0.7us of
    # issuing-engine time, ordering matches consumption: first h65 x-chunk,
    # params, rest of x interleaved with adjacency spans, masks last.
    xTr = []
    for ct in range(2):
        xf = big.tile([128, N], dt.bfloat16, tag=f"xTr{ct}")
        xTr.append(xf)
    waug = []
    wal = []
    xo = []

    def _params():
        for ct in range(2):
            wg = big.tile([128, WC], dt.bfloat16, tag=f"waug{ct}",
                          name=f"waug{ct}")
            nc.sync.dma_start(out=wg[:], in_=Waug[ct * 128:(ct + 1) * 128, :])
            waug.append(wg)
        for ct in range(2):
            wl = big.tile([128, HEADS], dt.bfloat16, tag=f"wal{ct}",
                          name=f"wal{ct}")
            nc.sync.dma_start(out=wl[:], in_=Wal[ct * 128:(ct + 1) * 128, :])
            wal.append(wl)
            t = big.tile([128, NB], dt.bfloat16, tag=f"xoT{ct}",
                         name=f"xoT{ct}")
            nc.sync.dma_start(out=t[:], in_=xoT[ct * 128:(ct + 1) * 128, :])
            xo.append(t)
    adjT_big = big.tile([128, JT * NB], dt.bfloat16, tag="adjT_big")
    # packed 16-bit masks only for the jt%4==3 tiles (the DVE AND path)
    NG = JT // 4
    adjm_big = big.tile([128, NG * NB // 2], dt.int32, tag="adjm_big")
    adjT = [adjT_big[:, jt * NB:(jt + 1) * NB] for jt in range(JT)]
    adjm = [adjm_big[:, g * NB // 2:(g + 1) * NB // 2] for g in range(NG)]
    # fp8 adjacency copy for the GpSimd mask multiplies (0/1 exact in fp8;
    # answers whether the GP multiply is element-rate or byte-rate)
    adjT8_big = big.tile([128, JT * NB], dt.float8e4, tag="adjT8_big")
    adjT8 = [adjT8_big[:, jt * NB:(jt + 1) * NB] for jt in range(JT)]

    def x_chunk(c, ct, lo=0, hi=1024):
        nc.sync.dma_start(
            out=xTr[ct][:, c * 1024 + lo:c * 1024 + hi],
            in_=xT[ct * 128:(ct + 1) * 128, c * 1024 + lo:c * 1024 + hi])

    def adj_span(lo, hi):
        n = hi - lo
        nc.sync.dma_start(
            out=adjT_big[:, lo * NB:hi * NB]
            .rearrange("p (j i) -> p j i", j=n),
            in_=adjbT[lo * 128:hi * 128, :]
            .rearrange("(j p) i -> p j i", p=128))

    def adjm_span():
        # rows 384 + 512*g + p (g in 0..7): the jt%4==3 row-tiles only
        RW = NB // 2
        nc.sync.dma_start(
            out=adjm_big[:].rearrange("p (g i) -> p g i", g=NG),
            in_=bass.AP(
                tensor=adjM.tensor,
                offset=adjM.offset + 3 * 128 * RW,
                ap=[[RW, 128], [512 * RW, NG], [1, RW]],
            ))

    # first half-chunks + waug land before the PE warmup ends so the h65
    # loop starts immediately
    x_chunk(0, 0, 0, 512)
    x_chunk(0, 1, 0, 512)
    _params()
    x_chunk(0, 0, 512, 1024)
    x_chunk(0, 1, 512, 1024)
    adj_span(0, 4)
    x_chunk(1, 0)
    x_chunk(1, 1)
    adj_span(4, 8)
    adj_span(8, 16)
    x_chunk(2, 0)
    x_chunk(2, 1)
    adj_span(16, 24)
    x_chunk(3, 0)
    x_chunk(3, 1)
    adj_span(24, 32)
    adjm_span()
    adjT8v = io["adjT8"]
    nc.sync.dma_start(
        out=adjT8_big[:].rearrange("p (j i) -> p j i", j=JT),
        in_=adjT8v.rearrange("(j p) i -> p j i", p=128))

    ht = []
    # er pack (col h*JT+jt) and ar = exp(er) pack (fp32 scalar source)
    erp = big.tile([128, HEADS * JT], dt.float32, tag="erp")
    erpv = erp[:].rearrange("p (h j) -> p h j", h=HEADS)
    arp = big.tile([128, HEADS * JT], dt.float32, tag="arp")
    arpv = arp[:].rearrange("p (h j) -> p h j", h=HEADS)
    arbr = big.tile([128, N], dt.bfloat16, tag="arbr")
    drhs = big.tile([128, NB], dt.bfloat16, tag="drhs")
    al_colsT = big.tile([128, IT * HEADS], dt.float32, tag="al_colsT")
    p1sb = []
    P1D = 6                 # P1 deferral behind h65

    with tc.tile_pool(name="psh", bufs=3, space="PSUM") as psh, \
         tc.tile_pool(name="pf", bufs=1, space="PSUM") as pf:
        # PE warmup while the first DMA chunks land: HAM clock up before
        # the real matmuls start.
        warm = psh.tile([128, WC], dt.float32, tag="hps")
        for _ in range(36):
            pe_mm(nc.tensor.matmul(warm[:, 0:64], lhsT=idb[:],
                                   rhs=idb[:, 0:64], start=True, stop=True))
        # trigger the ACT table load early (scalar queue is free here)
        scr = big.tile([1, 1], dt.float32, tag="scr")
        nc.scalar.activation(scr[:], bias_b[0:1, 0:1], Act.Exp)

        # el side is emitted inside the h65 loop (after nt 3) so the PE
        # does not stall on the xoT DMA right after warmup
        elp = pf.tile([HEADS, NB], dt.float32, tag="elp")
        al_sb = big.tile([HEADS, NB], dt.float32, tag="al_sb")

        def emit_el():
            for ct in range(2):
                pe_mm(nc.tensor.matmul(elp[:], lhsT=wal[ct][:],
                                       rhs=xo[ct][:],
                                       start=(ct == 0), stop=(ct == 1)))
            bl_b = big.tile([HEADS, NB], dt.bfloat16, tag="bl_b")
            al_b = big.tile([HEADS, NB], dt.bfloat16, tag="al_b")
            nc.scalar.activation(al_sb[:], elp[:], Act.Exp)
            nc.scalar.activation(bl_b[:], elp[:], Act.Exp, scale=0.2)
            nc.vector.tensor_copy(al_b[:], al_sb[:])
            # drhs rows at the PE row-group bases: 32h = bl_h, 32h+1 = al_h
            for h in range(HEADS):
                nc.gpsimd.dma_start(out=drhs[32 * h:32 * h + 1, :],
                                    in_=bl_b[h:h + 1, :])
                nc.gpsimd.dma_start(out=drhs[32 * h + 1:32 * h + 2, :],
                                    in_=al_b[h:h + 1, :])

        # ---- merged h65 + P1 loop ------------------------------------------
        po1f = [pf.tile([128, HEADS * C65], dt.float32, tag=f"po1f_{it}",
                        name=f"po1f_{it}") for it in range(IT)]

        def emit_arh(j):
            # ar-scaled h65 (SBUF-read scales - PSUM reads pay ~2x on DVE)
            a = arhp.tile([128, HEADS * C65], dt.bfloat16, tag="arh",
                          name=f"arh_{j}")
            for h in range(HEADS):
                sc = arpv[:, h, j:j + 1]
                dst = a[:, h * C65:(h + 1) * C65]
                src = ht[j][:, h * C65:(h + 1) * C65]
                if h < 2:
                    nc.vector.tensor_scalar_mul(dst, src, sc)
                else:
                    nc.scalar.activation(dst, src, Act.Copy, scale=sc)
            return a

        def p1_mm(j, a):
            for it in range(IT):
                pe_mm(nc.tensor.matmul(
                    po1f[it][:], lhsT=adjT[j][:, it * 128:(it + 1) * 128],
                    rhs=a[:], start=(j == 0), stop=(j == JT - 1),
                ))

        arh_q = []

        def h65_mm(nt):
            hps = psh.tile([128, WC], dt.float32, tag="hps",
                           name=f"hps{nt}")
            for ct in range(2):
                pe_mm(nc.tensor.matmul(
                    hps[:], lhsT=xTr[ct][:, nt * 128:(nt + 1) * 128],
                    rhs=waug[ct][:], start=(ct == 0), stop=(ct == 1),
                ))
            return hps

        def h65_ew(nt, hps):
            t = big.tile([128, HEADS * C65], dt.bfloat16, tag=f"h65_{nt}",
                         name=f"h65_{nt}")
            hr = t[:].rearrange("p (h c) -> p h c", c=C65)
            hpr = hps[:, 0:HC].rearrange("p (h c) -> p h c", c=OUT_C)
            if nt % 2 == 0:
                nc.vector.tensor_copy(hr[:, :, 0:OUT_C], hpr[:, :, :])
            else:
                nc.scalar.copy(hr[:, :, 0:OUT_C], hpr[:, :, :])
            nc.vector.memset(hr[:, :, OUT_C], 1.0)
            nc.vector.tensor_copy(erpv[:, :, nt], hps[:, HC:WC])
            nc.scalar.activation(arpv[:, :, nt], hps[:, HC:WC], Act.Exp)
            ht.append(t)
            arh_q.append(emit_arh(nt))

        # h65 emitted in pairs so its waits amortize over two tiles, P1
        # batched per pair as well (one wait covers 8 matmuls)
        for np_ in range(JT // 2):
            a, b = 2 * np_, 2 * np_ + 1
            ha = h65_mm(a)
            hb = h65_mm(b)
            h65_ew(a, ha)
            h65_ew(b, hb)
            if a == 2:
                emit_el()
            if b >= P1D + 1:
                p1_mm(b - P1D - 1, arh_q[b - P1D - 1])
                p1_mm(b - P1D, arh_q[b - P1D])
        for j in range(JT - P1D, JT):
            p1_mm(j, arh_q[j])

        # po1f -> SBUF copies (al scaling is fused into the epilogue STT)
        for it in range(IT):
            t = big.tile([128, HEADS * C65], dt.float32, tag=f"p1sb_{it}",
                         name=f"p1sb_{it}")
            if it % 2 == 0:
                nc.scalar.copy(t[:], po1f[it][:])
            else:
                nc.vector.tensor_copy(t[:], po1f[it][:])
            p1sb.append(t)

        # ---- glue: d-matmul lhsT rows [br; -ar] per head --------------------
        arb16 = big.tile([128, HEADS * JT], dt.bfloat16, tag="arb16")
        brb16 = big.tile([128, HEADS * JT], dt.bfloat16, tag="brb16")
        nc.vector.tensor_scalar_mul(arb16[:], arp[:], -1.0)
        nc.scalar.activation(brb16[:], erp[:], Act.Exp, scale=0.2)
        arT_ps = psh.tile([128, 128], dt.bfloat16, tag="hps")
        pe_mm(nc.tensor.transpose(arT_ps[:], arb16[:], idb[:]))
        arT_sb = big.tile([128, 128], dt.bfloat16, tag="arT_sb")
        nc.vector.tensor_copy(arT_sb[:], arT_ps[:])
        brT_ps = psh.tile([128, 128], dt.bfloat16, tag="hps")
        pe_mm(nc.tensor.transpose(brT_ps[:], brb16[:], idb[:]))
        brT_sb = big.tile([128, 128], dt.bfloat16, tag="brT_sb")
        nc.vector.tensor_copy(brT_sb[:], brT_ps[:])
        for h in range(HEADS):
            nc.gpsimd.dma_start(
                out=arbr[32 * h:32 * h + 1, :],
                in_=brT_sb[h * JT:(h + 1) * JT, :])
            nc.gpsimd.dma_start(
                out=arbr[32 * h + 1:32 * h + 2, :],
                in_=arT_sb[h * JT:(h + 1) * JT, :])
        # al columns for the epilogue (gpsimd queue is free by now)
        for it in range(IT):
            for h in range(HEADS):
                nc.gpsimd.dma_start(
                    out=al_colsT[:, it * HEADS + h:it * HEADS + h + 1],
                    in_=al_sb[h:h + 1, it * 128:(it + 1) * 128])

    # ---- main loop: two head passes -----------------------------------------
    # Pipeline per iteration k (emission order = engine queue order):
    #   PE:  d pair (k+2)  |  po2Y(k)  |  po2X(k-1)
    #   ACT: relu(k+1)   DVE: fused relu+mask STT(k+1)   GP: mask mult(k+1)
    # Every producer runs >=1 iteration before its consumer or bank reuse,
    # so no engine waits inside the steady state.
    osbs = {}
    with tc.tile_pool(name="dps", bufs=3, space="PSUM") as dps, \
         tc.tile_pool(name="pacc", bufs=1, space="PSUM") as pacc:

        def mk_hpass(hp):
            # closure-factory per head pair so hp1's pipeline preamble can
            # be emitted before hp0's drain (the PE chews hp1's d pairs
            # while GpSimd finishes hp0's last masks)
            hX, hY = 2 * hp, 2 * hp + 1
            st = {"dq": {}, "rY": [None] * JT, "rX": [None] * JT,
                  "ny": 0, "nx": 0}

            def alloc_acc():
                st["po2X"] = pacc.tile([C65, NB], dt.float32, tag="po2X",
                                       name=f"po2X{hp}")
                st["po2Y"] = pacc.tile([C65, NB], dt.float32, tag="po2Y",
                                       name=f"po2Y{hp}")

            def emit_d(j):
                dA = dps.tile([128, NB], dt.float32, tag="dA",
                              name=f"dA{hp}_{j}")
                dB = dps.tile([128, NB], dt.float32, tag="dB",
                              name=f"dB{hp}_{j}")
                for h, dst in ((hX, dA), (hY, dB)):
                    pe_mm(nc.tensor.matmul(
                        dst[:],
                        lhsT=arbr[32 * h:32 * h + 2, j * 128:(j + 1) * 128],
                        rhs=drhs[32 * h:32 * h + 2, :],
                        start=True, stop=True,
                        tile_position=(32 * h, 0),
                    ))
                st["dq"][j] = (dA, dB)

            def emit_ew(j):
                # elementwise chain for tile j (inputs ran >=1 iter ago)
                dA, dB = st["dq"][j]
                rd = rdp.tile([128, NB], dt.bfloat16, tag="rd",
                              name=f"rd{hp}_{j}")
                nc.scalar.activation(rd[:], dA[:], Act.Relu)
                ry = rYp.tile([128, NB], dt.bfloat16, tag="rY",
                              name=f"rY{hp}_{j}")
                nc.vector.scalar_tensor_tensor(
                    out=ry[:], in0=dB[:], scalar=0.0,
                    in1=adjT[j], op0=Alu.max, op1=Alu.mult)
                st["rY"][j] = ry
                rx = rXp.tile([128, NB], dt.bfloat16, tag="rX",
                              name=f"rX{hp}_{j}")
                if j % 4 != 3:
                    nc.gpsimd.tensor_mul(rx[:], rd[:], adjT8[j])
                else:
                    nc.vector.tensor_tensor(
                        rx[:].bitcast(dt.int32), rd[:].bitcast(dt.int32),
                        adjm[j // 4], op=Alu.bitwise_and)
                st["rX"][j] = rx

            def agg_y(j):
                pe_mm(nc.tensor.matmul(
                    st["po2Y"][:], lhsT=ht[j][:, hY * C65:(hY + 1) * C65],
                    rhs=st["rY"][j][:], start=(j == 0), stop=(j == JT - 1),
                ))

            def agg_x(j):
                pe_mm(nc.tensor.matmul(
                    st["po2X"][:], lhsT=ht[j][:, hX * C65:(hX + 1) * C65],
                    rhs=st["rX"][j][:], start=(j == 0), stop=(j == JT - 1),
                ))

            def flush_y(upto):
                # newest-first: the first matmul's wait covers the rest of
                # the batch, and a long agg run lets the PE pipeline settle
                js = list(range(st["ny"], upto))
                if not js:
                    return
                if st["ny"] == 0:
                    agg_y(0)
                    js = js[1:][::-1]
                else:
                    js = js[::-1]
                for j in js:
                    agg_y(j)
                st["ny"] = upto

            def flush_x(upto):
                js = list(range(st["nx"], upto))
                if not js:
                    return
                if st["nx"] == 0:
                    agg_x(0)
                    js = js[1:][::-1]
                else:
                    js = js[::-1]
                for j in js:
                    agg_x(j)
                st["nx"] = upto

            def preamble():
                for j in range(4):
                    emit_d(j)
                emit_ew(0)
                emit_ew(1)

            def body():
                # unroll-2, reversed in-pair order, aggregations deferred:
                # the d pair's semaphore waits (fresher ticks) cover every
                # aggregation wait, so agg batches of 8 issue back-to-back
                for K in range(JT // 2):
                    a, b = 2 * K, 2 * K + 1
                    if b + 4 < JT:
                        emit_d(b + 4)
                    if a + 4 < JT:
                        emit_d(a + 4)
                    if a + 2 < JT:
                        emit_ew(a + 2)
                    if b + 2 < JT:
                        emit_ew(b + 2)
                    if K % 2 == 1:
                        flush_y(b - 2 + 1)
                        flush_x(b - 4 + 1)

            def drain():
                flush_y(JT - 1)
                agg_y(JT - 1)
                flush_x(JT - 1)
                agg_x(JT - 1)
                # evacuate accumulators (bf16 suffices for the residual)
                oX = tr.tile([C65, NB], dt.bfloat16, tag="oX",
                             name=f"oX{hp}")
                nc.scalar.copy(oX[:], st["po2X"][:])
                oY = tr.tile([C65, NB], dt.bfloat16, tag="oY",
                             name=f"oY{hp}")
                nc.vector.tensor_copy(oY[:], st["po2Y"][:])
                osbs[hX] = oX
                osbs[hY] = oY

            st["fns"] = (alloc_acc, preamble, body, drain)
            return st

        s0 = mk_hpass(0)
        s1 = mk_hpass(1)
        s0["fns"][0]()          # alloc hp0 accumulators
        # keep-warm fillers bridge the glue gap at hpass-0 entry
        for k in range(12):
            pe_mm(nc.tensor.matmul(
                s0["po2X"][:, 0:OUT_C], lhsT=idb[:, 0:C65],
                rhs=idb[:, 0:OUT_C], start=True, stop=True))
        s0["fns"][1]()          # hp0 preamble
        s0["fns"][2]()          # hp0 body
        s1["fns"][1]()          # hp1 preamble BEFORE hp0 drain
        s0["fns"][3]()          # hp0 drain + evacuation
        s1["fns"][0]()          # alloc hp1 accumulators (after hp0 evac)
        s1["fns"][2]()          # hp1 body
        s1["fns"][3]()          # hp1 drain

    # ---- tail epilogue: transpose residual, combine with al-scaled P1 ------
    # all 8 PSUM banks are free now: transposes run one full row-block
    # ahead of the DVE combine chain
    with tc.tile_pool(name="pep", bufs=8, space="PSUM") as pep:
        ptq = {}

        def tr_it(it):
            for h in range(HEADS):
                pt = pep.tile([128, C65], dt.bfloat16, tag="pt",
                              name=f"pt{it}_{h}")
                pe_mm(nc.tensor.transpose(
                    pt[:], osbs[h][:, it * 128:(it + 1) * 128],
                    idb[0:C65, 0:C65]))
                ptq[(it, h)] = pt

        def comb_it(it):
            ot = tr.tile([128, HC], dt.float32, tag="ot", name=f"ot{it}")
            for h in range(HEADS):
                alc = al_colsT[:, it * HEADS + h:it * HEADS + h + 1]
                nd = tr.tile([128, C65], dt.float32, tag="nd",
                             name=f"nd{it}_{h}")
                nc.vector.scalar_tensor_tensor(
                    out=nd[:], in0=p1sb[it][:, h * C65:(h + 1) * C65],
                    scalar=alc, in1=ptq[(it, h)][:], op0=Alu.mult,
                    op1=Alu.add,
                )
                rec = tr.tile([128, 1], dt.float32, tag="rec",
                              name=f"rec{it}_{h}")
                nc.vector.reciprocal(rec[:], nd[:, OUT_C:C65])
                nc.vector.scalar_tensor_tensor(
                    out=ot[:, h * OUT_C:(h + 1) * OUT_C],
                    in0=nd[:, 0:OUT_C], scalar=rec[:],
                    in1=bias_b[:, h * OUT_C:(h + 1) * OUT_C],
                    op0=Alu.mult, op1=Alu.add,
                )
            nc.sync.dma_start(out=out[it * 128:(it + 1) * 128, :], in_=ot[:])

        tr_it(0)
        for it in range(1, IT):
            tr_it(it)
            comb_it(it - 1)
        comb_it(IT - 1)


def build():
    from contextlib import ExitStack
    import concourse.bacc as bacc
    import concourse.tile as tile
    from concourse import mybir

    dt = mybir.dt
    nc = bacc.Bacc("TRN2", target_bir_lowering=False, debug=False,
                   num_devices=NCORES)
    io = {
        "xT": nc.dram_tensor("xT", [IN_C, N], dt.bfloat16, kind="ExternalInput").ap(),
        "xoT": nc.dram_tensor("xoT", [IN_C, NB], dt.bfloat16, kind="ExternalInput").ap(),
        "adjbT": nc.dram_tensor("adjbT", [N, NB], dt.bfloat16, kind="ExternalInput").ap(),
        "adjT8": nc.dram_tensor("adjT8", [N, NB], dt.float8e4, kind="ExternalInput").ap(),
        "adjmask": nc.dram_tensor("adjmask", [N, NB // 2], dt.int32, kind="ExternalInput").ap(),
        "Waug": nc.dram_tensor("Waug", [IN_C, WC], dt.bfloat16, kind="ExternalInput").ap(),
        "Wal": nc.dram_tensor("Wal", [IN_C, HEADS], dt.bfloat16, kind="ExternalInput").ap(),
        "bias": nc.dram_tensor("bias", [HC], dt.float32, kind="ExternalInput").ap(),
        "out": nc.dram_tensor("out", [NB, HC], dt.float32, kind="ExternalOutput").ap(),
    }
    with tile.TileContext(nc) as tc:
        with ExitStack() as ctx:
            _emit(ctx, tc, nc, io)
    nc.compile()
    return nc


def make_in_maps(x, adj, W, att_l, att_r, bias):
    import ml_dtypes
    bf16 = ml_dtypes.bfloat16
    x = np.asarray(x, np.float32)
    adj = np.asarray(adj, np.int32)
    W = np.asarray(W, np.float32)
    att_l = np.asarray(att_l, np.float32)
    att_r = np.asarray(att_r, np.float32)
    bias = np.asarray(bias, np.float32)
    xT_b = np.ascontiguousarray(x.T.astype(bf16))
    Wr = W.reshape(IN_C, HEADS, OUT_C)
    Wal_ = np.ascontiguousarray(
        np.einsum("khc,hc->kh", Wr, att_l).astype(bf16))
    War = np.einsum("khc,hc->kh", Wr, att_r)
    Waug_b = np.ascontiguousarray(
        np.concatenate([W, War], axis=1).astype(bf16))
    adj_b = adj.astype(bf16)
    in_maps = []
    for m in range(NCORES):
        sl = slice(m * NB, (m + 1) * NB)
        adjbT = np.ascontiguousarray(adj_b[sl].T)
        mask16 = np.where(adjbT != 0, np.uint16(0xFFFF), np.uint16(0))
        adjmask = np.ascontiguousarray(mask16).view(np.int32)
        in_maps.append({
            "xT": xT_b,
            "adjT8": np.ascontiguousarray(
                adjbT.astype(ml_dtypes.float8_e4m3)),
            "xoT": np.ascontiguousarray(x[sl].T.astype(bf16)),
            "adjbT": adjbT,
            "adjmask": adjmask,
            "Waug": Waug_b,
            "Wal": Wal_,
            "bias": bias,
        })
    return in_maps


def _install_ntff_shim():
    import sys, types
    if "antenv.axon_hooks" in sys.modules:
        return
    from trn_agent_boot.trn_boot import _ntff_profile_via_ctypes
    hook = _ntff_profile_via_ctypes("/opt/axon/libaxon_pjrt.so")
    mod = types.ModuleType("antenv.axon_hooks")
    mod.get_axon_ntff_profile_hook = lambda: hook
    mod.set_axon_ntff_profile_hook = lambda h: None
    sys.modules["antenv.axon_hooks"] = mod


def kernel(x, adj, W, att_l, att_r, bias):
    from concourse.bass_utils import run_bass_kernel_spmd

    if "nc" not in _compiled:
        _compiled["nc"] = build()
    nc = _compiled["nc"]
    in_maps = make_in_maps(x, adj, W, att_l, att_r, bias)
    kwargs = {}
    if TRACE:
        _install_ntff_shim()
        kwargs["trace"] = True
    res = run_bass_kernel_spmd(nc, in_maps, core_ids=list(range(NCORES)), **kwargs)
    LAST_RESULTS["exec_time_ns"] = res.exec_time_ns
    LAST_RESULTS["mean_exec_time_ns"] = res.mean_exec_time_ns
    LAST_RESULTS["res"] = res
    return np.concatenate([res.results[m]["out"] for m in range(NCORES)], axis=0)
